# revision 47
# baseline (speedup 1.0000x reference)
"""AntiSymmetric GNN (2x AntiSymmetricConv + linear layers + log_softmax)
distributed Bass kernel for 8 TRN2 NeuronCores.

Strategy:
  - Nodes sharded by destination across 8 cores (12500/core, padded 12544).
  - Edges partitioned by destination core; per core sorted by
    (source-chunk, dest-window); aggregation = dma_gather of pre-scaled
    source features (dinv[src] * (h @ phi.T)) from an AllGathered table,
    then one-hot scatter matmuls accumulating per 128-dest window.
  - gcn norm factorizes: agg[c] = dinv[c] * (sum_e T[src_e] + T[c]),
    T = dinv*xw; the self-loop term T[c] is injected into the PSUM chain
    by an identity matmul; the antisymmetric term h@awT is folded into
    the same chain as (h/dinv[c]) @ awT so one dinv scale finishes z.
  - Tables are AllGathered in 4 window-aligned chunks (<=25600 rows so
    gather indices fit int16); t1 chunks fire inside phase A, t2 chunks
    (bf16, zero-padded to 256B rows) fire as soon as their quarter of
    windows retires.
  - SWDGE desc-gen is the wall: window-major loop, the 4 quarter
    segments of each window run on SWDGE queues 0-3 (queue q = Q7 core
    pair q) for 4x parallel descriptor generation; per-window epilogues
    (layer-2 prep, log_softmax) ride between gathers so vector/scalar
    load stays smooth and nothing serializes at phase boundaries.
"""

import numpy as np
import ml_dtypes

N = 100_000
F_IN = 256
HID = 128
C = 32
EPS = 0.1
GAMMA = 0.1

NCORES = 8
SHARD = 12_500
PADN = 12_544            # 98 * 128
W = 98                   # windows per core
QW = [25, 25, 24, 24]    # windows per quarter (window-aligned chunks)
QROWS = [3200, 3200, 3072, 3072]
QSTART = [0, 3200, 6400, 9472]
QWSTART = [0, 25, 50, 74]
MAX_SEG = 1024           # SWDGE ring cap per dma_gather call

_CACHE = {}


def _host_prep(x, lin1_w, lin1_b, lin2_w, lin2_b, W1, phi1_w, b1, W2, phi2_w, b2,
               edge_index):
    bf16 = ml_dtypes.bfloat16
    row = edge_index[0].astype(np.int64)
    col = edge_index[1].astype(np.int64)

    # degrees INCLUDE self loops (reference appends them)
    deg = (np.bincount(col, minlength=N) + 1).astype(np.float32)
    dinv = 1.0 / np.sqrt(deg)

    # source -> (chunk, int16 index into chunk table)
    ks = row // SHARD
    i_s = row % SHARD
    wloc = i_s // 128
    q_s = np.where(wloc < 25, 0, np.where(wloc < 50, 1, np.where(wloc < 74, 2, 3)))
    pos = i_s - np.asarray(QSTART)[q_s]
    idx16_all = ks * np.asarray(QROWS)[q_s] + pos

    k_dst = col // SHARD

    cores = []
    L = np.zeros((NCORES, 4 * W), np.int64)
    for k in range(NCORES):
        m = k_dst == k
        r_idx = idx16_all[m]
        c_loc = col[m] - k * SHARD
        key = q_s[m] * W + c_loc // 128
        order = np.argsort(key, kind="stable")
        cores.append((key[order], r_idx[order],
                      (c_loc % 128)[order].astype(np.float32)))
        L[k] = np.bincount(cores[k][0], minlength=4 * W)

    LMAX = np.maximum(L.max(axis=0), (L.max(axis=0) > 0).astype(np.int64))
    assert LMAX.max() <= MAX_SEG, f"segment too large: {LMAX.max()}"

    cols16 = (LMAX + 15) // 16
    tiles = (LMAX + 127) // 128
    seg_off16 = np.zeros(4 * W + 1, np.int64)
    np.cumsum(cols16, out=seg_off16[1:])
    seg_offt = np.zeros(4 * W + 1, np.int64)
    np.cumsum(tiles, out=seg_offt[1:])

    per_core = []
    for k in range(NCORES):
        key_s, idx_s, cl_s = cores[k]
        idx_arr = np.full(int(seg_off16[-1]) * 16, -1, np.int16)
        colv_arr = np.full(int(seg_offt[-1]) * 128, -1.0, np.float32)
        starts_src = np.zeros(4 * W + 1, np.int64)
        np.cumsum(L[k], out=starts_src[1:])
        seg_of = np.repeat(np.arange(4 * W), L[k])
        off_in = np.arange(len(idx_s)) - starts_src[seg_of]
        idx_arr[seg_off16[seg_of] * 16 + off_in] = idx_s.astype(np.int16)
        colv_arr[seg_offt[seg_of] * 128 + off_in] = cl_s
        # pad [L_k, lmax) of each segment with cycled copies of its own real
        # indices (S zeroes them) -- spreads pad reads across HBM instead of
        # hammering row 0
        for sgi in np.nonzero(LMAX > L[k])[0]:
            b = int(seg_off16[sgi]) * 16
            lk = int(L[k][sgi])
            lm = int(LMAX[sgi])
            if lk > 0:
                idx_arr[b + lk:b + lm] = np.resize(idx_arr[b:b + lk], lm - lk)
            else:
                idx_arr[b + lk:b + lm] = 0
        per_core.append((idx_arr, colv_arr))

    def wrap_idx(arr):
        a16 = arr.reshape(-1, 16).T
        return np.ascontiguousarray(np.tile(a16, (8, 1)))

    in_maps = []
    for k in range(NCORES):
        idx_arr, colv_arr = per_core[k]
        xs = np.zeros((PADN, F_IN), np.float32)
        xs[:SHARD] = x[k * SHARD:(k + 1) * SHARD]
        xT = np.ascontiguousarray(xs.T).astype(bf16)
        dvk = np.zeros(PADN, np.float32)
        dvk[:SHARD] = dinv[k * SHARD:(k + 1) * SHARD]
        dvk_i = np.zeros(PADN, np.float32)
        dvk_i[:SHARD] = 1.0 / dvk[:SHARD]
        im = {
            "xT": xT,
            "dinv_w": np.ascontiguousarray(dvk.reshape(W, 128).T),
            "dinvi_w": np.ascontiguousarray(dvk_i.reshape(W, 128).T),
            "lin1T": np.ascontiguousarray(lin1_w.T).astype(bf16),
            "phi1T": np.ascontiguousarray(phi1_w.T).astype(bf16),
            "aw1T": np.ascontiguousarray(
                (W1 - W1.T - GAMMA * np.eye(HID, dtype=np.float32)).T).astype(bf16),
            "lin2T": np.ascontiguousarray(lin2_w.T).astype(bf16),
            # fused layer-2 weights: t2-row = dinv*(h1p@M2), aw-term = h1p@M3
            "M2": np.ascontiguousarray(
                lin2_w.T @ phi2_w.T).astype(bf16),
            "M3": np.ascontiguousarray(
                lin2_w.T @ (W2 - W2.T - GAMMA * np.eye(C, dtype=np.float32)).T
            ).astype(bf16),
        }
        for q in range(4):
            s16 = slice(int(seg_off16[q * W]) * 16, int(seg_off16[(q + 1) * W]) * 16)
            st = slice(int(seg_offt[q * W]) * 128, int(seg_offt[(q + 1) * W]) * 128)
            im[f"idx{q}"] = wrap_idx(idx_arr[s16])
            im[f"colv{q}"] = np.ascontiguousarray(
                colv_arr[st].reshape(-1, 128).T.astype(bf16))
        in_maps.append(im)

    aw2 = W2 - W2.T - GAMMA * np.eye(C, dtype=np.float32)
    biases = {
        "blin1": np.broadcast_to(lin1_b, (128, HID)).astype(np.float32).copy(),
        "bconv1": np.broadcast_to(b1, (128, HID)).astype(np.float32).copy(),
        "blin2": np.broadcast_to(lin2_b, (128, C)).astype(np.float32).copy(),
        "bconv2": np.broadcast_to(b2, (128, C)).astype(np.float32).copy(),
        "b2phi": np.broadcast_to(lin2_b @ phi2_w.T, (128, C))
            .astype(np.float32).copy(),
        "b2aw": np.broadcast_to(lin2_b @ aw2.T, (128, C))
            .astype(np.float32).copy(),
    }
    use_bias = {name: bool(np.any(arr)) for name, arr in biases.items()}
    for name, used in use_bias.items():
        if used:
            for im in in_maps:
                im[name] = biases[name]

    import os
    meta = {
        "LMAX": LMAX,
        "tiles": tiles, "cols16": cols16,
        "seg_off16": seg_off16, "seg_offt": seg_offt,
        "use_bias": use_bias,
        "phases": int(os.environ.get("KERNEL_PHASES", "5")),
    }
    return in_maps, meta


def _build_graph(meta):
    import concourse.bass as bass
    import concourse.mybir as mybir
    import concourse.tile as tile
    from concourse import bacc
    from concourse.masks import make_identity
    from contextlib import ExitStack

    dt = mybir.dt
    Alu = mybir.AluOpType
    Act = mybir.ActivationFunctionType
    LMAX = meta["LMAX"]
    tiles = meta["tiles"]
    cols16 = meta["cols16"]
    seg_off16 = meta["seg_off16"]
    seg_offt = meta["seg_offt"]
    use_bias = meta["use_bias"]
    phases = meta.get("phases", 5)

    nc = bacc.Bacc("TRN2", target_bir_lowering=False, num_swdge_queues=4)

    xT = nc.declare_dram_parameter("xT", [F_IN, PADN], dt.bfloat16, isOutput=False)
    dinv_w = nc.declare_dram_parameter("dinv_w", [128, W], dt.float32, isOutput=False)
    dinvi_w = nc.declare_dram_parameter("dinvi_w", [128, W], dt.float32,
                                        isOutput=False)
    lin1T = nc.declare_dram_parameter("lin1T", [F_IN, HID], dt.bfloat16, isOutput=False)
    phi1T = nc.declare_dram_parameter("phi1T", [HID, HID], dt.bfloat16, isOutput=False)
    aw1T = nc.declare_dram_parameter("aw1T", [HID, HID], dt.bfloat16, isOutput=False)
    lin2T = nc.declare_dram_parameter("lin2T", [HID, C], dt.bfloat16, isOutput=False)
    M2 = nc.declare_dram_parameter("M2", [HID, C], dt.bfloat16, isOutput=False)
    M3 = nc.declare_dram_parameter("M3", [HID, C], dt.bfloat16, isOutput=False)
    idx_p, colv_p = [], []
    for q in range(4):
        n16 = int(seg_off16[(q + 1) * W] - seg_off16[q * W])
        nt = int(seg_offt[(q + 1) * W] - seg_offt[q * W])
        idx_p.append(nc.declare_dram_parameter(f"idx{q}", [128, n16], dt.int16,
                                               isOutput=False))
        colv_p.append(nc.declare_dram_parameter(f"colv{q}", [128, nt],
                                                dt.bfloat16, isOutput=False))
    bias_p = {}
    for name, shape in [("blin1", [128, HID]), ("bconv1", [128, HID]),
                        ("blin2", [128, C]), ("bconv2", [128, C])]:
        if use_bias[name]:
            bias_p[name] = nc.declare_dram_parameter(name, shape, dt.float32,
                                                     isOutput=False)
    out_p = nc.declare_dram_parameter("out", [PADN, C], dt.float32, isOutput=True)

    t1q_in = [nc.dram_tensor(f"t1in{q}", [QROWS[q], HID], dt.bfloat16)
              for q in range(4)]
    t1q_tab = [nc.dram_tensor(f"t1tab{q}", [NCORES * QROWS[q], HID], dt.bfloat16,
                              addr_space="Shared") for q in range(4)]
    t2q_in = [nc.dram_tensor(f"t2in{q}", [QROWS[q], HID], dt.bfloat16)
              for q in range(4)]
    t2q_tab = [nc.dram_tensor(f"t2tab{q}", [NCORES * QROWS[q], HID], dt.bfloat16,
                              addr_space="Shared") for q in range(4)]

    rg = [list(range(NCORES))]
    GB = 4

    def wq(w):
        return 0 if w < 25 else (1 if w < 50 else (2 if w < 74 else 3))

    with tile.TileContext(nc) as tc, ExitStack() as top:
        const = top.enter_context(tc.tile_pool(name="const", bufs=1))
        big = top.enter_context(tc.tile_pool(name="big", bufs=1))
        tmp_pool = top.enter_context(tc.tile_pool(name="tmp", bufs=2))
        icp = top.enter_context(tc.tile_pool(name="icp", bufs=1))

        # resident idx/colv for all 4 quarters, shared by both layers
        idx_sb, colv_sb = [], []
        for q in range(4):
            n16 = int(seg_off16[(q + 1) * W] - seg_off16[q * W])
            ntq = int(seg_offt[(q + 1) * W] - seg_offt[q * W])
            it = icp.tile([128, n16], dt.int16, tag=f"idx{q}")
            nc.sync.dma_start(it[:], idx_p[q][:])
            idx_sb.append(it)
            ct = icp.tile([128, ntq], dt.bfloat16, tag=f"colv{q}")
            nc.sync.dma_start(ct[:], colv_p[q][:])
            colv_sb.append(ct)

        lin1T_sb = const.tile([128, 2, HID], dt.bfloat16)
        nc.sync.dma_start(lin1T_sb[:], lin1T[:].rearrange("(t p) j -> p t j", p=128))
        phi1T_sb = const.tile([128, HID], dt.bfloat16)
        nc.sync.dma_start(phi1T_sb[:], phi1T[:])
        aw1T_sb = const.tile([128, HID], dt.bfloat16)
        nc.sync.dma_start(aw1T_sb[:], aw1T[:])
        lin2T_sb = const.tile([128, C], dt.bfloat16)
        nc.sync.dma_start(lin2T_sb[:], lin2T[:])
        M2_sb = const.tile([128, C], dt.bfloat16)
        nc.sync.dma_start(M2_sb[:], M2[:])
        M3_sb = const.tile([128, C], dt.bfloat16)
        nc.sync.dma_start(M3_sb[:], M3[:])
        dinv_sb = const.tile([128, W], dt.float32)
        nc.sync.dma_start(dinv_sb[:], dinv_w[:])
        dinvi_sb = const.tile([128, W], dt.float32)
        nc.sync.dma_start(dinvi_sb[:], dinvi_w[:])
        bias_sb = {}
        for name, p in bias_p.items():
            t = const.tile(list(p.shape), dt.float32)
            nc.sync.dma_start(t[:], p[:])
            bias_sb[name] = t

        zerosA = const.tile([128, HID], dt.float32)
        nc.vector.memset(zerosA[:], 0.0)
        iota_i = const.tile([128, 128], dt.int32)
        nc.gpsimd.iota(iota_i[:], pattern=[[1, 128]], base=0, channel_multiplier=0)
        iota_bf = const.tile([128, 128], dt.bfloat16)
        nc.vector.tensor_copy(iota_bf[:], iota_i[:])
        ident = const.tile([128, 128], dt.bfloat16)
        make_identity(nc, ident[:])
        MAXT = int(meta["tiles"].max()) if len(meta["tiles"]) else 1
        iota_t = const.tile([128, MAXT, 128], dt.bfloat16)
        for jj in range(MAXT):
            nc.vector.tensor_copy(iota_t[:, jj, :], iota_bf[:])

        h1 = big.tile([128, W, HID], dt.bfloat16, tag="h1")
        t1all = big.tile([128, W, HID], dt.bfloat16, tag="t1all")
        t2all = big.tile([128, W, 128], dt.bfloat16, tag="t2all")
        h2 = big.tile([128, W, C], dt.float32, tag="h2")
        h1pT_all = big.tile([128, W, 128], dt.bfloat16, tag="h1pT_all")
        agg2 = big.tile([128, W, C], dt.float32, tag="agg2")

        nc.vector.memset(t2all[:], 0.0)

        MAXTILES = int(tiles.max()) if len(tiles) else 1

        # ===== Phase A: h1 = relu(x@lin1+b); T1 = dinv * (h1@phi1T) =====
        # t1 AllGather for each quarter fires as soon as its chunk is done
        with tc.tile_pool(name="xq", bufs=2) as xqp, \
             tc.tile_pool(name="psA", bufs=2, space="PSUM") as psA, \
             tc.tile_pool(name="psAT", bufs=2, space="PSUM") as psAT:
            for q in range(4):
                c0 = QWSTART[q] * 128
                cw = QW[q] * 128
                xq_sb = xqp.tile([128, 2, cw], dt.bfloat16, tag="xq")
                nc.sync.dma_start(
                    xq_sb[:],
                    xT[:, c0:c0 + cw].rearrange("(t p) c -> p t c", p=128))
                for wi in range(QW[q]):
                    w = QWSTART[q] + wi
                    ph = psA.tile([128, HID], dt.float32, tag="ph")
                    nc.tensor.matmul(ph[:], xq_sb[:, 0, wi * 128:(wi + 1) * 128],
                                     lin1T_sb[:, 0, :], start=True, stop=False)
                    nc.tensor.matmul(ph[:], xq_sb[:, 1, wi * 128:(wi + 1) * 128],
                                     lin1T_sb[:, 1, :], start=False, stop=True)
                    if "blin1" in bias_sb:
                        t = tmp_pool.tile([128, HID], dt.float32, tag="tA")
                        nc.vector.tensor_tensor(t[:], ph[:], bias_sb["blin1"][:],
                                                op=Alu.add)
                        nc.scalar.activation(h1[:, w, :], t[:], Act.Relu)
                    else:
                        nc.scalar.activation(h1[:, w, :], ph[:], Act.Relu)
                    pt = psAT.tile([128, 128], dt.bfloat16, tag="pt")
                    nc.tensor.transpose(pt[:], h1[:, w, :], ident[:])
                    h1Tw = tmp_pool.tile([128, 128], dt.bfloat16, tag="h1Tw")
                    nc.vector.tensor_copy(h1Tw[:], pt[:])
                    pT = psA.tile([128, HID], dt.float32, tag="pT1")
                    nc.tensor.matmul(pT[:], h1Tw[:], phi1T_sb[:],
                                     start=True, stop=True)
                    nc.vector.tensor_tensor(
                        t1all[:, w, :], pT[:],
                        dinv_sb[:, w:w + 1].broadcast_to([128, HID]),
                        op=Alu.mult)
                nc.sync.dma_start(
                    t1q_in[q][:].rearrange("(w p) f -> p w f", p=128),
                    t1all[:, QWSTART[q]:QWSTART[q] + QW[q], :])
                nc.gpsimd.collective_compute(
                    "AllGather", Alu.bypass, replica_groups=rg,
                    ins=[t1q_in[q][:].opt()], outs=[t1q_tab[q][:].opt()])

        # ===== Aggregation (both layers), window-major =====
        nc.vector.memset(h2[:], 0.0)
        if phases >= 2:
         with tc.tile_pool(name="gp", bufs=16) as gp, \
             tc.tile_pool(name="sp", bufs=8) as sp, \
             tc.tile_pool(name="psP", bufs=3, space="PSUM") as psP, \
             tc.tile_pool(name="psT", bufs=2, space="PSUM") as psT, \
             tc.tile_pool(name="psS", bufs=2, space="PSUM") as psS, \
             tc.tile_pool(name="psW", bufs=1, space="PSUM") as psW:
            for i in range(16):
                z = gp.tile([128, MAXTILES, HID], dt.bfloat16, tag="g")
                nc.vector.memset(z[:], 0.0)

            def seg_matmuls(w, pseg, table, fw, selfrows, close):
                """one window's psum chain: self-loop + 4 quarter segments"""
                segs = [q for q in range(4) if int(tiles[q * W + w]) > 0]
                nc.tensor.matmul(pseg[:, 0:fw], ident[:], selfrows,
                                 start=True, stop=(close and not segs))
                for qi, q in enumerate(segs):
                    s = q * W + w
                    nt = int(tiles[s])
                    lmax = int(LMAX[s])
                    o16 = int(seg_off16[s]) - int(seg_off16[q * W])
                    ot = int(seg_offt[s]) - int(seg_offt[q * W])
                    g = gp.tile([128, nt, HID], dt.bfloat16, tag="g")
                    nc.gpsimd.dma_gather(
                        g[:], table[q][:],
                        idx_sb[q][:, o16:o16 + int(cols16[s])],
                        lmax, lmax, HID, queue_num=q)
                    S = sp.tile([128, nt, 128], dt.bfloat16, tag="S")
                    nc.vector.tensor_tensor(
                        S[:],
                        iota_t[:, 0:nt, :],
                        colv_sb[q][:, ot:ot + nt].unsqueeze(2)
                            .broadcast_to([128, nt, 128]),
                        op=Alu.is_equal)
                    last = close and qi == len(segs) - 1
                    for j in range(nt):
                        nc.tensor.matmul(pseg[:, 0:fw], S[:, j, :], g[:, j, 0:fw],
                                         start=False,
                                         stop=(last and j == nt - 1))

            def d_step(w, pseg):
                """layer-2 prep for window w; closes pseg with the h@awT term"""
                # h1s = h1 / dinv[c]; (h1s.T @ aw1T) accumulated into pseg so
                # that one dinv scale yields z = h1@aw1T + dinv*(agg+T1self)
                h1s = tmp_pool.tile([128, HID], dt.bfloat16, tag="h1s")
                nc.scalar.activation(h1s[:], h1[:, w, :], Act.Copy,
                                     scale=dinvi_sb[:, w:w + 1])
                pt = psT.tile([128, 128], dt.bfloat16, tag="pt")
                nc.tensor.transpose(pt[:], h1s[:], ident[:])
                h1sT = tmp_pool.tile([128, 128], dt.bfloat16, tag="h1sT")
                nc.scalar.copy(h1sT[:], pt[:])
                nc.tensor.matmul(pseg[:], h1sT[:], aw1T_sb[:],
                                 start=False, stop=True)
                th = tmp_pool.tile([128, HID], dt.float32, tag="th")
                if "bconv1" in bias_sb:
                    pre = tmp_pool.tile([128, HID], dt.float32, tag="pre")
                    nc.scalar.activation(pre[:], pseg[:], Act.Copy,
                                         scale=dinv_sb[:, w:w + 1])
                    nc.vector.tensor_tensor(pre[:], pre[:], bias_sb["bconv1"][:],
                                            op=Alu.add)
                    nc.scalar.activation(th[:], pre[:], Act.Tanh)
                else:
                    nc.scalar.activation(th[:], pseg[:], Act.Tanh,
                                         scale=dinv_sb[:, w:w + 1])
                h1p = tmp_pool.tile([128, HID], dt.bfloat16, tag="h1p")
                nc.vector.scalar_tensor_tensor(
                    h1p[:], th[:], EPS, h1[:, w, :], op0=Alu.mult, op1=Alu.add)
                pt2 = psT.tile([128, 128], dt.bfloat16, tag="pt")
                nc.tensor.transpose(pt2[:], h1p[:], ident[:])
                nc.scalar.copy(h1pT_all[:, w, :], pt2[:])
                ph2 = psS.tile([128, C], dt.float32, tag="ps2")
                nc.tensor.matmul(ph2[:], h1pT_all[:, w, :], lin2T_sb[:],
                                 start=True, stop=True)
                if "blin2" in bias_sb:
                    nc.vector.tensor_tensor(h2[:, w, :], ph2[:],
                                            bias_sb["blin2"][:], op=Alu.add)
                else:
                    nc.vector.tensor_copy(h2[:, w, :], ph2[:])
                pT2 = psS.tile([128, C], dt.float32, tag="ps2")
                nc.tensor.matmul(pT2[:], h1pT_all[:, w, :], M2_sb[:],
                                 start=True, stop=True)
                if "b2phi" in bias_sb:
                    tb = tmp_pool.tile([128, C], dt.float32, tag="tb2")
                    nc.vector.tensor_tensor(tb[:], pT2[:], bias_sb["b2phi"][:],
                                            op=Alu.add)
                    nc.scalar.activation(t2all[:, w, 0:C], tb[:], Act.Copy,
                                         scale=dinv_sb[:, w:w + 1])
                else:
                    nc.scalar.activation(t2all[:, w, 0:C], pT2[:], Act.Copy,
                                         scale=dinv_sb[:, w:w + 1])
                q = wq(w)
                if w - QWSTART[q] == QW[q] - 1:
                    nc.sync.dma_start(
                        t2q_in[q][:].rearrange("(w p) f -> p w f", p=128),
                        t2all[:, QWSTART[q]:QWSTART[q] + QW[q], :])
                    nc.gpsimd.collective_compute(
                        "AllGather", Alu.bypass, replica_groups=rg,
                        ins=[t2q_in[q][:].opt()], outs=[t2q_tab[q][:].opt()])

            gstate = {}

            def g_group(w0, gw):
                paw4 = psW.tile([128, HID], dt.float32, tag="paw")
                for wi in range(gw):
                    nc.tensor.matmul(paw4[:, wi * C:(wi + 1) * C],
                                     h1pT_all[:, w0 + wi, :], M3_sb[:],
                                     start=True, stop=True)
                a1 = tmp_pool.tile([128, GB, C], dt.float32, tag="a1g")
                nc.vector.tensor_tensor(
                    a1[:, 0:gw, :], agg2[:, w0:w0 + gw, :],
                    dinv_sb[:, w0:w0 + gw].unsqueeze(2)
                        .broadcast_to([128, gw, C]),
                    op=Alu.mult)
                pre = tmp_pool.tile([128, GB, C], dt.float32, tag="preg")
                nc.vector.tensor_tensor(
                    pre[:, 0:gw, :], a1[:, 0:gw, :],
                    paw4[:, 0:gw * C].rearrange("p (t c) -> p t c", c=C),
                    op=Alu.add)
                for bn in ("bconv2", "b2aw"):
                    if bn in bias_sb:
                        nc.vector.tensor_tensor(
                            pre[:, 0:gw, :], pre[:, 0:gw, :],
                            bias_sb[bn][:].unsqueeze(1)
                                .broadcast_to([128, gw, C]),
                            op=Alu.add)
                th = tmp_pool.tile([128, GB, C], dt.float32, tag="thg")
                nc.scalar.activation(th[:, 0:gw, :], pre[:, 0:gw, :], Act.Tanh)
                h2p = tmp_pool.tile([128, GB, C], dt.float32, tag="h2pg")
                nc.vector.scalar_tensor_tensor(
                    h2p[:, 0:gw, :], th[:, 0:gw, :], EPS, h2[:, w0:w0 + gw, :],
                    op0=Alu.mult, op1=Alu.add)
                negmax = tmp_pool.tile([128, GB, 1], dt.float32, tag="nmg")
                nc.vector.tensor_reduce(negmax[:, 0:gw, :], h2p[:, 0:gw, :],
                                        axis=mybir.AxisListType.X,
                                        op=Alu.max, negate=True)
                sub = tmp_pool.tile([128, GB, C], dt.float32, tag="subg")
                nc.vector.tensor_tensor(
                    sub[:, 0:gw, :], h2p[:, 0:gw, :],
                    negmax[:, 0:gw, :].broadcast_to([128, gw, C]), op=Alu.add)
                e = tmp_pool.tile([128, GB, C], dt.float32, tag="eg")
                nc.scalar.activation(e[:, 0:gw, :], sub[:, 0:gw, :], Act.Exp)
                ssum = tmp_pool.tile([128, GB, 1], dt.float32, tag="ssg")
                nc.vector.tensor_reduce(ssum[:, 0:gw, :], e[:, 0:gw, :],
                                        axis=mybir.AxisListType.X, op=Alu.add)
                lse = tmp_pool.tile([128, GB, 1], dt.float32, tag="lseg")
                nc.scalar.activation(lse[:, 0:gw, :], ssum[:, 0:gw, :], Act.Ln)
                nc.vector.tensor_tensor(
                    agg2[:, w0:w0 + gw, :], sub[:, 0:gw, :],
                    lse[:, 0:gw, :].broadcast_to([128, gw, C]), op=Alu.subtract)

            def g_step(w):
                if phases >= 5 and (w % GB == GB - 1 or w == W - 1):
                    w0 = (w // GB) * GB
                    g_group(w0, w - w0 + 1)

            # layer 1, window-major; per-window epilogues lag LAG windows so
            # their cross-engine round trips never stall the gather cadence
            LAG = 2
            open_pseg = {}
            for w in range(W):
                pseg = psP.tile([128, HID], dt.float32, tag="pseg")
                open_pseg[w] = pseg
                seg_matmuls(w, pseg, t1q_tab, HID, t1all[:, w, :],
                            close=(phases < 3))
                if phases >= 3 and w >= LAG:
                    d_step(w - LAG, open_pseg.pop(w - LAG))
            if phases >= 3:
                for w in range(W - LAG, W):
                    d_step(w, open_pseg.pop(w))

            # layer 2, quarter-major: quarter q's gathers depend only on the
            # t2 AllGather chunk that fired at ~(q+1)/4 of layer 1, so no
            # stall waiting for the last chunk; agg2 accumulates in SBUF
            if phases >= 4:
                nc.vector.tensor_copy(agg2[:], t2all[:, :, 0:C])
                for q in range(4):
                    for w in range(W):
                        s = q * W + w
                        nt = int(tiles[s])
                        if nt == 0:
                            if q == 3 and w >= LAG:
                                g_step(w - LAG)
                            continue
                        lmax = int(LMAX[s])
                        o16 = int(seg_off16[s]) - int(seg_off16[q * W])
                        ot = int(seg_offt[s]) - int(seg_offt[q * W])
                        g = gp.tile([128, nt, HID], dt.bfloat16, tag="g")
                        nc.gpsimd.dma_gather(
                            g[:], t2q_tab[q][:],
                            idx_sb[q][:, o16:o16 + int(cols16[s])],
                            lmax, lmax, HID, queue_num=w % 4)
                        S = sp.tile([128, nt, 128], dt.bfloat16, tag="S")
                        nc.vector.tensor_tensor(
                            S[:],
                            iota_t[:, 0:nt, :],
                            colv_sb[q][:, ot:ot + nt].unsqueeze(2)
                                .broadcast_to([128, nt, 128]),
                            op=Alu.is_equal)
                        pseg = psP.tile([128, HID], dt.float32, tag="pseg")
                        for j in range(nt):
                            nc.tensor.matmul(pseg[:, 0:C], S[:, j, :],
                                             g[:, j, 0:C],
                                             start=(j == 0), stop=(j == nt - 1))
                        nc.vector.tensor_tensor(agg2[:, w, :], agg2[:, w, :],
                                                pseg[:, 0:C], op=Alu.add)
                        if q == 3 and w >= LAG:
                            g_step(w - LAG)
                    if q == 3:
                        for w in range(W - LAG, W):
                            g_step(w)

        nc.sync.dma_start(out_p[:].rearrange("(w p) c -> p w c", p=128), agg2[:])

    nc.compile()
    return nc


def kernel(**inputs):
    from concourse.bass_utils import run_bass_kernel_spmd

    inp = {k: np.asarray(v) for k, v in inputs.items()}
    in_maps, meta = _host_prep(**inp)

    key = ("graph", tuple(meta["LMAX"].tolist()),
           tuple(sorted(meta["use_bias"].items())), meta["phases"])
    if key not in _CACHE:
        _CACHE[key] = _build_graph(meta)
    nc = _CACHE[key]

    import os
    trace = bool(int(os.environ.get("KERNEL_TRACE", "0")))
    res = run_bass_kernel_spmd(nc, in_maps, list(range(NCORES)), trace=trace,
                               tmpdir=os.environ.get("KERNEL_TRACE_DIR"))
    global LAST_EXEC_NS
    LAST_EXEC_NS = res.exec_time_ns

    out = np.concatenate([res.results[k]["out"][:SHARD] for k in range(NCORES)], 0)
    return out.astype(np.float32)


LAST_EXEC_NS = None


# revision 48
# speedup vs baseline: 1.1500x; 1.1500x over previous
"""AntiSymmetric GNN (2x AntiSymmetricConv + linear layers + log_softmax)
distributed Bass kernel for 8 TRN2 NeuronCores.

Strategy:
  - Nodes sharded by destination across 8 cores (12500/core, padded 12544).
  - Edges partitioned by destination core; per core sorted by
    (source-chunk, dest-window); aggregation = dma_gather of pre-scaled
    source features (dinv[src] * (h @ phi.T)) from an AllGathered table,
    then one-hot scatter matmuls accumulating per 128-dest window.
  - gcn norm factorizes: agg[c] = dinv[c] * (sum_e T[src_e] + T[c]),
    T = dinv*xw; the self-loop term T[c] is injected into the PSUM chain
    by an identity matmul; the antisymmetric term h@awT is folded into
    the same chain as (h/dinv[c]) @ awT so one dinv scale finishes z.
  - Tables are AllGathered in 4 window-aligned chunks (<=25600 rows so
    gather indices fit int16); t1 chunks fire inside phase A, t2 chunks
    (bf16, zero-padded to 256B rows) fire as soon as their quarter of
    windows retires.
  - SWDGE desc-gen is the wall: window-major loop, the 4 quarter
    segments of each window run on SWDGE queues 0-3 (queue q = Q7 core
    pair q) for 4x parallel descriptor generation; per-window epilogues
    (layer-2 prep, log_softmax) ride between gathers so vector/scalar
    load stays smooth and nothing serializes at phase boundaries.
"""

import numpy as np
import ml_dtypes

N = 100_000
F_IN = 256
HID = 128
C = 32
EPS = 0.1
GAMMA = 0.1

NCORES = 8
SHARD = 12_500
PADN = 12_544            # 98 * 128
W = 98                   # windows per core
QW = [25, 25, 24, 24]    # windows per quarter (window-aligned chunks)
QROWS = [3200, 3200, 3072, 3072]
QSTART = [0, 3200, 6400, 9472]
QWSTART = [0, 25, 50, 74]
MAX_SEG = 1024           # SWDGE ring cap per dma_gather call

_CACHE = {}


def _host_prep(x, lin1_w, lin1_b, lin2_w, lin2_b, W1, phi1_w, b1, W2, phi2_w, b2,
               edge_index):
    bf16 = ml_dtypes.bfloat16
    row = edge_index[0].astype(np.int64)
    col = edge_index[1].astype(np.int64)

    # degrees INCLUDE self loops (reference appends them)
    deg = (np.bincount(col, minlength=N) + 1).astype(np.float32)
    dinv = 1.0 / np.sqrt(deg)

    # source -> (chunk, int16 index into chunk table)
    ks = row // SHARD
    i_s = row % SHARD
    wloc = i_s // 128
    q_s = np.where(wloc < 25, 0, np.where(wloc < 50, 1, np.where(wloc < 74, 2, 3)))
    pos = i_s - np.asarray(QSTART)[q_s]
    idx16_all = ks * np.asarray(QROWS)[q_s] + pos

    k_dst = col // SHARD

    cores = []
    L = np.zeros((NCORES, 4 * W), np.int64)
    for k in range(NCORES):
        m = k_dst == k
        r_idx = idx16_all[m]
        c_loc = col[m] - k * SHARD
        key = q_s[m] * W + c_loc // 128
        order = np.argsort(key, kind="stable")
        cores.append((key[order], r_idx[order],
                      (c_loc % 128)[order].astype(np.float32)))
        L[k] = np.bincount(cores[k][0], minlength=4 * W)

    LMAX = np.maximum(L.max(axis=0), (L.max(axis=0) > 0).astype(np.int64))
    assert LMAX.max() <= MAX_SEG, f"segment too large: {LMAX.max()}"

    cols16 = (LMAX + 15) // 16
    tiles = (LMAX + 127) // 128
    seg_off16 = np.zeros(4 * W + 1, np.int64)
    np.cumsum(cols16, out=seg_off16[1:])
    seg_offt = np.zeros(4 * W + 1, np.int64)
    np.cumsum(tiles, out=seg_offt[1:])

    per_core = []
    for k in range(NCORES):
        key_s, idx_s, cl_s = cores[k]
        idx_arr = np.full(int(seg_off16[-1]) * 16, -1, np.int16)
        colv_arr = np.full(int(seg_offt[-1]) * 128, -1.0, np.float32)
        starts_src = np.zeros(4 * W + 1, np.int64)
        np.cumsum(L[k], out=starts_src[1:])
        seg_of = np.repeat(np.arange(4 * W), L[k])
        off_in = np.arange(len(idx_s)) - starts_src[seg_of]
        idx_arr[seg_off16[seg_of] * 16 + off_in] = idx_s.astype(np.int16)
        colv_arr[seg_offt[seg_of] * 128 + off_in] = cl_s
        # pad [L_k, lmax) of each segment with cycled copies of its own real
        # indices (S zeroes them) -- spreads pad reads across HBM instead of
        # hammering row 0
        for sgi in np.nonzero(LMAX > L[k])[0]:
            b = int(seg_off16[sgi]) * 16
            lk = int(L[k][sgi])
            lm = int(LMAX[sgi])
            if lk > 0:
                idx_arr[b + lk:b + lm] = np.resize(idx_arr[b:b + lk], lm - lk)
            else:
                idx_arr[b + lk:b + lm] = 0
        per_core.append((idx_arr, colv_arr))

    def wrap_idx(arr):
        a16 = arr.reshape(-1, 16).T
        return np.ascontiguousarray(np.tile(a16, (8, 1)))

    in_maps = []
    for k in range(NCORES):
        idx_arr, colv_arr = per_core[k]
        xs = np.zeros((PADN, F_IN), np.float32)
        xs[:SHARD] = x[k * SHARD:(k + 1) * SHARD]
        xT = np.ascontiguousarray(xs.T).astype(bf16)
        dvk = np.zeros(PADN, np.float32)
        dvk[:SHARD] = dinv[k * SHARD:(k + 1) * SHARD]
        dvk_i = np.zeros(PADN, np.float32)
        dvk_i[:SHARD] = 1.0 / dvk[:SHARD]
        im = {
            "xT": xT,
            "dinv_w": np.ascontiguousarray(dvk.reshape(W, 128).T),
            "dinvi_w": np.ascontiguousarray(dvk_i.reshape(W, 128).T),
            "lin1T": np.ascontiguousarray(lin1_w.T).astype(bf16),
            "phi1T": np.ascontiguousarray(phi1_w.T).astype(bf16),
            "aw1T": np.ascontiguousarray(
                (W1 - W1.T - GAMMA * np.eye(HID, dtype=np.float32)).T).astype(bf16),
            "lin2T": np.ascontiguousarray(lin2_w.T).astype(bf16),
            # fused layer-2 weights: t2-row = dinv*(h1p@M2), aw-term = h1p@M3
            "M2": np.ascontiguousarray(
                lin2_w.T @ phi2_w.T).astype(bf16),
            "M3": np.ascontiguousarray(
                lin2_w.T @ (W2 - W2.T - GAMMA * np.eye(C, dtype=np.float32)).T
            ).astype(bf16),
        }
        for q in range(4):
            s16 = slice(int(seg_off16[q * W]) * 16, int(seg_off16[(q + 1) * W]) * 16)
            st = slice(int(seg_offt[q * W]) * 128, int(seg_offt[(q + 1) * W]) * 128)
            im[f"idx{q}"] = wrap_idx(idx_arr[s16])
            im[f"colv{q}"] = np.ascontiguousarray(
                colv_arr[st].reshape(-1, 128).T.astype(bf16))
        in_maps.append(im)

    aw2 = W2 - W2.T - GAMMA * np.eye(C, dtype=np.float32)
    biases = {
        "blin1": np.broadcast_to(lin1_b, (128, HID)).astype(np.float32).copy(),
        "bconv1": np.broadcast_to(b1, (128, HID)).astype(np.float32).copy(),
        "blin2": np.broadcast_to(lin2_b, (128, C)).astype(np.float32).copy(),
        "bconv2": np.broadcast_to(b2, (128, C)).astype(np.float32).copy(),
        "b2phi": np.broadcast_to(lin2_b @ phi2_w.T, (128, C))
            .astype(np.float32).copy(),
        "b2aw": np.broadcast_to(lin2_b @ aw2.T, (128, C))
            .astype(np.float32).copy(),
    }
    use_bias = {name: bool(np.any(arr)) for name, arr in biases.items()}
    for name, used in use_bias.items():
        if used:
            for im in in_maps:
                im[name] = biases[name]

    import os
    meta = {
        "LMAX": LMAX,
        "tiles": tiles, "cols16": cols16,
        "seg_off16": seg_off16, "seg_offt": seg_offt,
        "use_bias": use_bias,
        "phases": int(os.environ.get("KERNEL_PHASES", "5")),
    }
    return in_maps, meta


def _build_graph(meta):
    import concourse.bass as bass
    import concourse.mybir as mybir
    import concourse.tile as tile
    from concourse import bacc
    from concourse.masks import make_identity
    from contextlib import ExitStack

    dt = mybir.dt
    Alu = mybir.AluOpType
    Act = mybir.ActivationFunctionType
    LMAX = meta["LMAX"]
    tiles = meta["tiles"]
    cols16 = meta["cols16"]
    seg_off16 = meta["seg_off16"]
    seg_offt = meta["seg_offt"]
    use_bias = meta["use_bias"]
    phases = meta.get("phases", 5)

    nc = bacc.Bacc("TRN2", target_bir_lowering=False, num_swdge_queues=4)

    xT = nc.declare_dram_parameter("xT", [F_IN, PADN], dt.bfloat16, isOutput=False)
    dinv_w = nc.declare_dram_parameter("dinv_w", [128, W], dt.float32, isOutput=False)
    dinvi_w = nc.declare_dram_parameter("dinvi_w", [128, W], dt.float32,
                                        isOutput=False)
    lin1T = nc.declare_dram_parameter("lin1T", [F_IN, HID], dt.bfloat16, isOutput=False)
    phi1T = nc.declare_dram_parameter("phi1T", [HID, HID], dt.bfloat16, isOutput=False)
    aw1T = nc.declare_dram_parameter("aw1T", [HID, HID], dt.bfloat16, isOutput=False)
    lin2T = nc.declare_dram_parameter("lin2T", [HID, C], dt.bfloat16, isOutput=False)
    M2 = nc.declare_dram_parameter("M2", [HID, C], dt.bfloat16, isOutput=False)
    M3 = nc.declare_dram_parameter("M3", [HID, C], dt.bfloat16, isOutput=False)
    idx_p, colv_p = [], []
    for q in range(4):
        n16 = int(seg_off16[(q + 1) * W] - seg_off16[q * W])
        nt = int(seg_offt[(q + 1) * W] - seg_offt[q * W])
        idx_p.append(nc.declare_dram_parameter(f"idx{q}", [128, n16], dt.int16,
                                               isOutput=False))
        colv_p.append(nc.declare_dram_parameter(f"colv{q}", [128, nt],
                                                dt.bfloat16, isOutput=False))
    bias_p = {}
    for name, shape in [("blin1", [128, HID]), ("bconv1", [128, HID]),
                        ("blin2", [128, C]), ("bconv2", [128, C])]:
        if use_bias[name]:
            bias_p[name] = nc.declare_dram_parameter(name, shape, dt.float32,
                                                     isOutput=False)
    out_p = nc.declare_dram_parameter("out", [PADN, C], dt.float32, isOutput=True)

    t1q_in = [nc.dram_tensor(f"t1in{q}", [QROWS[q], HID], dt.bfloat16)
              for q in range(4)]
    t1q_tab = [nc.dram_tensor(f"t1tab{q}", [NCORES * QROWS[q], HID], dt.bfloat16,
                              addr_space="Shared") for q in range(4)]
    t2q_in = [nc.dram_tensor(f"t2in{q}", [QROWS[q], HID], dt.bfloat16)
              for q in range(4)]
    t2q_tab = [nc.dram_tensor(f"t2tab{q}", [NCORES * QROWS[q], HID], dt.bfloat16,
                              addr_space="Shared") for q in range(4)]

    rg = [list(range(NCORES))]
    GB = 4

    def wq(w):
        return 0 if w < 25 else (1 if w < 50 else (2 if w < 74 else 3))

    with tile.TileContext(nc) as tc, ExitStack() as top:
        const = top.enter_context(tc.tile_pool(name="const", bufs=1))
        big = top.enter_context(tc.tile_pool(name="big", bufs=1))
        tmp_pool = top.enter_context(tc.tile_pool(name="tmp", bufs=2))
        icp = top.enter_context(tc.tile_pool(name="icp", bufs=1))

        # resident idx/colv for all 4 quarters, shared by both layers
        idx_sb, colv_sb = [], []
        for q in range(4):
            n16 = int(seg_off16[(q + 1) * W] - seg_off16[q * W])
            ntq = int(seg_offt[(q + 1) * W] - seg_offt[q * W])
            it = icp.tile([128, n16], dt.int16, tag=f"idx{q}")
            nc.sync.dma_start(it[:], idx_p[q][:])
            idx_sb.append(it)
            ct = icp.tile([128, ntq], dt.bfloat16, tag=f"colv{q}")
            nc.sync.dma_start(ct[:], colv_p[q][:])
            colv_sb.append(ct)

        lin1T_sb = const.tile([128, 2, HID], dt.bfloat16)
        nc.sync.dma_start(lin1T_sb[:], lin1T[:].rearrange("(t p) j -> p t j", p=128))
        phi1T_sb = const.tile([128, HID], dt.bfloat16)
        nc.sync.dma_start(phi1T_sb[:], phi1T[:])
        aw1T_sb = const.tile([128, HID], dt.bfloat16)
        nc.sync.dma_start(aw1T_sb[:], aw1T[:])
        lin2T_sb = const.tile([128, C], dt.bfloat16)
        nc.sync.dma_start(lin2T_sb[:], lin2T[:])
        M2_sb = const.tile([128, C], dt.bfloat16)
        nc.sync.dma_start(M2_sb[:], M2[:])
        M3_sb = const.tile([128, C], dt.bfloat16)
        nc.sync.dma_start(M3_sb[:], M3[:])
        dinv_sb = const.tile([128, W], dt.float32)
        nc.sync.dma_start(dinv_sb[:], dinv_w[:])
        dinvi_sb = const.tile([128, W], dt.float32)
        nc.sync.dma_start(dinvi_sb[:], dinvi_w[:])
        bias_sb = {}
        for name, p in bias_p.items():
            t = const.tile(list(p.shape), dt.float32)
            nc.sync.dma_start(t[:], p[:])
            bias_sb[name] = t

        zerosA = const.tile([128, HID], dt.float32)
        nc.vector.memset(zerosA[:], 0.0)
        iota_i = const.tile([128, 128], dt.int32)
        nc.gpsimd.iota(iota_i[:], pattern=[[1, 128]], base=0, channel_multiplier=0)
        iota_bf = const.tile([128, 128], dt.bfloat16)
        nc.vector.tensor_copy(iota_bf[:], iota_i[:])
        ident = const.tile([128, 128], dt.bfloat16)
        make_identity(nc, ident[:])
        MAXT = int(meta["tiles"].max()) if len(meta["tiles"]) else 1
        iota_t = const.tile([128, MAXT, 128], dt.bfloat16)
        for jj in range(MAXT):
            nc.vector.tensor_copy(iota_t[:, jj, :], iota_bf[:])

        h1 = big.tile([128, W, HID], dt.bfloat16, tag="h1")
        t1all = big.tile([128, W, HID], dt.bfloat16, tag="t1all")
        t2all = big.tile([128, W, 128], dt.bfloat16, tag="t2all")
        h2 = big.tile([128, W, C], dt.float32, tag="h2")
        h1pT_all = big.tile([128, W, 128], dt.bfloat16, tag="h1pT_all")
        agg2 = big.tile([128, W, C], dt.float32, tag="agg2")

        nc.vector.memset(t2all[:], 0.0)

        MAXTILES = int(tiles.max()) if len(tiles) else 1

        # ===== Phase A: h1 = relu(x@lin1+b); T1 = dinv * (h1@phi1T) =====
        # t1 AllGather for each quarter fires as soon as its chunk is done
        with tc.tile_pool(name="xq", bufs=2) as xqp, \
             tc.tile_pool(name="psA", bufs=2, space="PSUM") as psA, \
             tc.tile_pool(name="psAT", bufs=2, space="PSUM") as psAT:
            for q in range(4):
                c0 = QWSTART[q] * 128
                cw = QW[q] * 128
                xq_sb = xqp.tile([128, 2, cw], dt.bfloat16, tag="xq")
                nc.sync.dma_start(
                    xq_sb[:],
                    xT[:, c0:c0 + cw].rearrange("(t p) c -> p t c", p=128))
                for wi in range(QW[q]):
                    w = QWSTART[q] + wi
                    ph = psA.tile([128, HID], dt.float32, tag="ph")
                    nc.tensor.matmul(ph[:], xq_sb[:, 0, wi * 128:(wi + 1) * 128],
                                     lin1T_sb[:, 0, :], start=True, stop=False)
                    nc.tensor.matmul(ph[:], xq_sb[:, 1, wi * 128:(wi + 1) * 128],
                                     lin1T_sb[:, 1, :], start=False, stop=True)
                    if "blin1" in bias_sb:
                        t = tmp_pool.tile([128, HID], dt.float32, tag="tA")
                        nc.vector.tensor_tensor(t[:], ph[:], bias_sb["blin1"][:],
                                                op=Alu.add)
                        nc.scalar.activation(h1[:, w, :], t[:], Act.Relu)
                    else:
                        nc.scalar.activation(h1[:, w, :], ph[:], Act.Relu)
                    pt = psAT.tile([128, 128], dt.bfloat16, tag="pt")
                    nc.tensor.transpose(pt[:], h1[:, w, :], ident[:])
                    h1Tw = tmp_pool.tile([128, 128], dt.bfloat16, tag="h1Tw")
                    nc.vector.tensor_copy(h1Tw[:], pt[:])
                    pT = psA.tile([128, HID], dt.float32, tag="pT1")
                    nc.tensor.matmul(pT[:], h1Tw[:], phi1T_sb[:],
                                     start=True, stop=True)
                    nc.vector.tensor_tensor(
                        t1all[:, w, :], pT[:],
                        dinv_sb[:, w:w + 1].broadcast_to([128, HID]),
                        op=Alu.mult)
                nc.sync.dma_start(
                    t1q_in[q][:].rearrange("(w p) f -> p w f", p=128),
                    t1all[:, QWSTART[q]:QWSTART[q] + QW[q], :])
                nc.gpsimd.collective_compute(
                    "AllGather", Alu.bypass, replica_groups=rg,
                    ins=[t1q_in[q][:].opt()], outs=[t1q_tab[q][:].opt()])

        # ===== Aggregation (both layers), window-major =====
        nc.vector.memset(h2[:], 0.0)
        if phases >= 2:
         with tc.tile_pool(name="gp", bufs=16) as gp, \
             tc.tile_pool(name="sp", bufs=8) as sp, \
             tc.tile_pool(name="psP", bufs=3, space="PSUM") as psP, \
             tc.tile_pool(name="psT", bufs=2, space="PSUM") as psT, \
             tc.tile_pool(name="psS", bufs=2, space="PSUM") as psS, \
             tc.tile_pool(name="psW", bufs=1, space="PSUM") as psW:
            for i in range(16):
                z = gp.tile([128, MAXTILES, HID], dt.bfloat16, tag="g")
                nc.vector.memset(z[:], 0.0)

            def seg_matmuls(w, pseg, table, fw, selfrows, close):
                """one window's psum chain: self-loop + 4 quarter segments"""
                segs = [q for q in range(4) if int(tiles[q * W + w]) > 0]
                nc.tensor.matmul(pseg[:, 0:fw], ident[:], selfrows,
                                 start=True, stop=(close and not segs))
                for qi, q in enumerate(segs):
                    s = q * W + w
                    nt = int(tiles[s])
                    lmax = int(LMAX[s])
                    o16 = int(seg_off16[s]) - int(seg_off16[q * W])
                    ot = int(seg_offt[s]) - int(seg_offt[q * W])
                    g = gp.tile([128, nt, HID], dt.bfloat16, tag="g")
                    nc.gpsimd.dma_gather(
                        g[:], table[q][:],
                        idx_sb[q][:, o16:o16 + int(cols16[s])],
                        lmax, lmax, HID, queue_num=(q + w) % 4)
                    S = sp.tile([128, nt, 128], dt.bfloat16, tag="S")
                    nc.vector.tensor_tensor(
                        S[:],
                        iota_t[:, 0:nt, :],
                        colv_sb[q][:, ot:ot + nt].unsqueeze(2)
                            .broadcast_to([128, nt, 128]),
                        op=Alu.is_equal)
                    last = close and qi == len(segs) - 1
                    for j in range(nt):
                        nc.tensor.matmul(pseg[:, 0:fw], S[:, j, :], g[:, j, 0:fw],
                                         start=False,
                                         stop=(last and j == nt - 1))

            def d_step(w, pseg):
                """layer-2 prep for window w; closes pseg with the h@awT term"""
                # h1s = h1 / dinv[c]; (h1s.T @ aw1T) accumulated into pseg so
                # that one dinv scale yields z = h1@aw1T + dinv*(agg+T1self)
                h1s = tmp_pool.tile([128, HID], dt.bfloat16, tag="h1s")
                nc.scalar.activation(h1s[:], h1[:, w, :], Act.Copy,
                                     scale=dinvi_sb[:, w:w + 1])
                pt = psT.tile([128, 128], dt.bfloat16, tag="pt")
                nc.tensor.transpose(pt[:], h1s[:], ident[:])
                h1sT = tmp_pool.tile([128, 128], dt.bfloat16, tag="h1sT")
                nc.scalar.copy(h1sT[:], pt[:])
                nc.tensor.matmul(pseg[:], h1sT[:], aw1T_sb[:],
                                 start=False, stop=True)
                th = tmp_pool.tile([128, HID], dt.float32, tag="th")
                if "bconv1" in bias_sb:
                    pre = tmp_pool.tile([128, HID], dt.float32, tag="pre")
                    nc.scalar.activation(pre[:], pseg[:], Act.Copy,
                                         scale=dinv_sb[:, w:w + 1])
                    nc.vector.tensor_tensor(pre[:], pre[:], bias_sb["bconv1"][:],
                                            op=Alu.add)
                    nc.scalar.activation(th[:], pre[:], Act.Tanh)
                else:
                    nc.scalar.activation(th[:], pseg[:], Act.Tanh,
                                         scale=dinv_sb[:, w:w + 1])
                h1p = tmp_pool.tile([128, HID], dt.bfloat16, tag="h1p")
                nc.vector.scalar_tensor_tensor(
                    h1p[:], th[:], EPS, h1[:, w, :], op0=Alu.mult, op1=Alu.add)
                pt2 = psT.tile([128, 128], dt.bfloat16, tag="pt")
                nc.tensor.transpose(pt2[:], h1p[:], ident[:])
                nc.scalar.copy(h1pT_all[:, w, :], pt2[:])
                ph2 = psS.tile([128, C], dt.float32, tag="ps2")
                nc.tensor.matmul(ph2[:], h1pT_all[:, w, :], lin2T_sb[:],
                                 start=True, stop=True)
                if "blin2" in bias_sb:
                    nc.vector.tensor_tensor(h2[:, w, :], ph2[:],
                                            bias_sb["blin2"][:], op=Alu.add)
                else:
                    nc.vector.tensor_copy(h2[:, w, :], ph2[:])
                pT2 = psS.tile([128, C], dt.float32, tag="ps2")
                nc.tensor.matmul(pT2[:], h1pT_all[:, w, :], M2_sb[:],
                                 start=True, stop=True)
                if "b2phi" in bias_sb:
                    tb = tmp_pool.tile([128, C], dt.float32, tag="tb2")
                    nc.vector.tensor_tensor(tb[:], pT2[:], bias_sb["b2phi"][:],
                                            op=Alu.add)
                    nc.scalar.activation(t2all[:, w, 0:C], tb[:], Act.Copy,
                                         scale=dinv_sb[:, w:w + 1])
                else:
                    nc.scalar.activation(t2all[:, w, 0:C], pT2[:], Act.Copy,
                                         scale=dinv_sb[:, w:w + 1])
                q = wq(w)
                if w - QWSTART[q] == QW[q] - 1:
                    nc.sync.dma_start(
                        t2q_in[q][:].rearrange("(w p) f -> p w f", p=128),
                        t2all[:, QWSTART[q]:QWSTART[q] + QW[q], :])
                    nc.gpsimd.collective_compute(
                        "AllGather", Alu.bypass, replica_groups=rg,
                        ins=[t2q_in[q][:].opt()], outs=[t2q_tab[q][:].opt()])

            gstate = {}

            def g_group(w0, gw):
                paw4 = psW.tile([128, HID], dt.float32, tag="paw")
                for wi in range(gw):
                    nc.tensor.matmul(paw4[:, wi * C:(wi + 1) * C],
                                     h1pT_all[:, w0 + wi, :], M3_sb[:],
                                     start=True, stop=True)
                a1 = tmp_pool.tile([128, GB, C], dt.float32, tag="a1g")
                nc.vector.tensor_tensor(
                    a1[:, 0:gw, :], agg2[:, w0:w0 + gw, :],
                    dinv_sb[:, w0:w0 + gw].unsqueeze(2)
                        .broadcast_to([128, gw, C]),
                    op=Alu.mult)
                pre = tmp_pool.tile([128, GB, C], dt.float32, tag="preg")
                nc.vector.tensor_tensor(
                    pre[:, 0:gw, :], a1[:, 0:gw, :],
                    paw4[:, 0:gw * C].rearrange("p (t c) -> p t c", c=C),
                    op=Alu.add)
                for bn in ("bconv2", "b2aw"):
                    if bn in bias_sb:
                        nc.vector.tensor_tensor(
                            pre[:, 0:gw, :], pre[:, 0:gw, :],
                            bias_sb[bn][:].unsqueeze(1)
                                .broadcast_to([128, gw, C]),
                            op=Alu.add)
                th = tmp_pool.tile([128, GB, C], dt.float32, tag="thg")
                nc.scalar.activation(th[:, 0:gw, :], pre[:, 0:gw, :], Act.Tanh)
                h2p = tmp_pool.tile([128, GB, C], dt.float32, tag="h2pg")
                nc.vector.scalar_tensor_tensor(
                    h2p[:, 0:gw, :], th[:, 0:gw, :], EPS, h2[:, w0:w0 + gw, :],
                    op0=Alu.mult, op1=Alu.add)
                negmax = tmp_pool.tile([128, GB, 1], dt.float32, tag="nmg")
                nc.vector.tensor_reduce(negmax[:, 0:gw, :], h2p[:, 0:gw, :],
                                        axis=mybir.AxisListType.X,
                                        op=Alu.max, negate=True)
                sub = tmp_pool.tile([128, GB, C], dt.float32, tag="subg")
                nc.vector.tensor_tensor(
                    sub[:, 0:gw, :], h2p[:, 0:gw, :],
                    negmax[:, 0:gw, :].broadcast_to([128, gw, C]), op=Alu.add)
                e = tmp_pool.tile([128, GB, C], dt.float32, tag="eg")
                nc.scalar.activation(e[:, 0:gw, :], sub[:, 0:gw, :], Act.Exp)
                ssum = tmp_pool.tile([128, GB, 1], dt.float32, tag="ssg")
                nc.vector.tensor_reduce(ssum[:, 0:gw, :], e[:, 0:gw, :],
                                        axis=mybir.AxisListType.X, op=Alu.add)
                lse = tmp_pool.tile([128, GB, 1], dt.float32, tag="lseg")
                nc.scalar.activation(lse[:, 0:gw, :], ssum[:, 0:gw, :], Act.Ln)
                nc.vector.tensor_tensor(
                    agg2[:, w0:w0 + gw, :], sub[:, 0:gw, :],
                    lse[:, 0:gw, :].broadcast_to([128, gw, C]), op=Alu.subtract)

            def g_step(w):
                if phases >= 5 and (w % GB == GB - 1 or w == W - 1):
                    w0 = (w // GB) * GB
                    g_group(w0, w - w0 + 1)

            # layer 1, window-major; per-window epilogues lag LAG windows so
            # their cross-engine round trips never stall the gather cadence
            LAG = 2
            open_pseg = {}
            for w in range(W):
                pseg = psP.tile([128, HID], dt.float32, tag="pseg")
                open_pseg[w] = pseg
                seg_matmuls(w, pseg, t1q_tab, HID, t1all[:, w, :],
                            close=(phases < 3))
                if phases >= 3 and w >= LAG:
                    d_step(w - LAG, open_pseg.pop(w - LAG))
            if phases >= 3:
                for w in range(W - LAG, W):
                    d_step(w, open_pseg.pop(w))

            # layer 2, quarter-major: quarter q's gathers depend only on the
            # t2 AllGather chunk that fired at ~(q+1)/4 of layer 1, so no
            # stall waiting for the last chunk; agg2 accumulates in SBUF
            if phases >= 4:
                nc.vector.tensor_copy(agg2[:], t2all[:, :, 0:C])
                for q in range(4):
                    for w in range(W):
                        s = q * W + w
                        nt = int(tiles[s])
                        if nt == 0:
                            if q == 3 and w >= LAG:
                                g_step(w - LAG)
                            continue
                        lmax = int(LMAX[s])
                        o16 = int(seg_off16[s]) - int(seg_off16[q * W])
                        ot = int(seg_offt[s]) - int(seg_offt[q * W])
                        g = gp.tile([128, nt, HID], dt.bfloat16, tag="g")
                        nc.gpsimd.dma_gather(
                            g[:], t2q_tab[q][:],
                            idx_sb[q][:, o16:o16 + int(cols16[s])],
                            lmax, lmax, HID, queue_num=w % 4)
                        S = sp.tile([128, nt, 128], dt.bfloat16, tag="S")
                        nc.vector.tensor_tensor(
                            S[:],
                            iota_t[:, 0:nt, :],
                            colv_sb[q][:, ot:ot + nt].unsqueeze(2)
                                .broadcast_to([128, nt, 128]),
                            op=Alu.is_equal)
                        pseg = psP.tile([128, HID], dt.float32, tag="pseg")
                        for j in range(nt):
                            nc.tensor.matmul(pseg[:, 0:C], S[:, j, :],
                                             g[:, j, 0:C],
                                             start=(j == 0), stop=(j == nt - 1))
                        nc.vector.tensor_tensor(agg2[:, w, :], agg2[:, w, :],
                                                pseg[:, 0:C], op=Alu.add)
                        if q == 3 and w >= LAG:
                            g_step(w - LAG)
                    if q == 3:
                        for w in range(W - LAG, W):
                            g_step(w)

        nc.sync.dma_start(out_p[:].rearrange("(w p) c -> p w c", p=128), agg2[:])

    nc.compile()
    return nc


def kernel(**inputs):
    from concourse.bass_utils import run_bass_kernel_spmd

    inp = {k: np.asarray(v) for k, v in inputs.items()}
    in_maps, meta = _host_prep(**inp)

    key = ("graph", tuple(meta["LMAX"].tolist()),
           tuple(sorted(meta["use_bias"].items())), meta["phases"])
    if key not in _CACHE:
        _CACHE[key] = _build_graph(meta)
    nc = _CACHE[key]

    import os
    trace = bool(int(os.environ.get("KERNEL_TRACE", "0")))
    res = run_bass_kernel_spmd(nc, in_maps, list(range(NCORES)), trace=trace,
                               tmpdir=os.environ.get("KERNEL_TRACE_DIR"))
    global LAST_EXEC_NS
    LAST_EXEC_NS = res.exec_time_ns

    out = np.concatenate([res.results[k]["out"][:SHARD] for k in range(NCORES)], 0)
    return out.astype(np.float32)


LAST_EXEC_NS = None
